# revision 1
# baseline (speedup 1.0000x reference)
"""DMR induction routing kernel for Trainium2 (Bass/Tile), 8-core data-parallel.

Problem: nn_DMRInduction. Full inputs:
  m [128, 768], q [256, 768], W [768, 765], b [765] -> out [256, 765] fp32.

Sharding: Q=256 split 8 ways (32 queries/core); m, W, b replicated.

Per-core layouts:
  - hat_m        [I=128, C*D=765]   (I on partitions)  - hv weights / final hv rhs
  - hmT aug      [D+1=154, I] per c (D on partitions)  - num/mdv weights;
      row 153 holds -mean_c(m) so the num matmul computes the centered
      correlation numerator directly (sum_d xm*tq = sum_d m*tq - mum*colsum).
  - tq, v        [D, C*Q=160] as two tiles [128,160] + [34,160]
      (tqB row 32 carries colsum for the augmented num matmul and the
       yn2 correction; vB rows 25..33 stay zero so mdv stays uncentered).
  - routing state a, p, dsp [I=128, C*Q=160].
  - final hat_v  [Q=32, C*D=765] -> squash -> contiguous DMA out.
"""
import os
import sys

for _p in ("/opt/trn_rl_repo", "/root/.axon_site/_ro/trn_rl_repo"):
    if os.path.isdir(_p) and _p not in sys.path:
        sys.path.insert(0, _p)

import numpy as np
import concourse.bass as bass
import concourse.bacc as bacc
import concourse.mybir as mybir
import concourse.tile as tile
from concourse.bass_utils import run_bass_kernel_spmd

F32 = mybir.dt.float32
# Matmul input dtype. float32 is exact (final scale-relative err ~2e-5);
# float32r uses the fast PE path (1 cyc/row at N>=256 vs 4) and cuts the
# projection phase ~14us, at ~2.5e-4 scale-relative output error. The
# rest of the kernel is dependency-latency-bound, so the dtype only
# affects the projection matmuls. Default to exact.
DT = getattr(mybir.dt, os.environ.get("KERNEL_MM_DT", "float32"))

NCORES = 8
I = 128         # memory capsules
C = 5           # capsule classes
D = 153         # dim per capsule
CD = C * D      # 765
K = 768         # input dim
KC = K // 128   # 6 contraction chunks
QL = 32         # queries per core
CQ = C * QL     # 160
NPAD = 768      # W padded to 768 cols so fp32r matmuls stream N>=256
EPS = 1e-8
AX = mybir.AxisListType.X
MUL = mybir.AluOpType.mult
ADD = mybir.AluOpType.add
SUB = mybir.AluOpType.subtract


def build(with_bias: bool, dbg: bool = False):
    nc = bacc.Bacc("TRN2", target_bir_lowering=False, debug=False)

    mT_d = nc.dram_tensor("mT", [K, I], F32, kind="ExternalInput")
    qT_d = nc.dram_tensor("qT", [K, QL], F32, kind="ExternalInput")
    W_d = nc.dram_tensor("Wp", [K, NPAD], F32, kind="ExternalInput")
    b_d = nc.dram_tensor("b", [1, CD], F32, kind="ExternalInput")
    eye_d = nc.dram_tensor("eye", [128, 128], F32, kind="ExternalInput")
    ones_d = nc.dram_tensor("onesv", [128, 1], F32, kind="ExternalInput")
    zeros_d = nc.dram_tensor("zerosv", [128, 644], F32, kind="ExternalInput")
    onesc_d = nc.dram_tensor("onescv", [34, 1], F32, kind="ExternalInput")
    out_d = nc.dram_tensor("out", [QL, CD], F32, kind="ExternalOutput")
    dbg_d = {}
    if dbg:
        for nm, shp in [("hatm", [128, CD]), ("hatq", [QL, CD]), ("tqA0", [128, CQ]),
                        ("p1", [128, CQ]), ("a1", [128, CQ]), ("p2", [128, CQ]),
                        ("a2", [128, CQ]), ("p3", [128, CQ]), ("mTc1d", [128, C * 128]),
                        ("mTc2d", [34, C * 128]), ("tqB0", [34, CQ])]:
            dbg_d[nm] = nc.dram_tensor("dbg_" + nm, shp, F32, kind="ExternalOutput")

    with tile.TileContext(nc) as tc:
        with (
            tc.tile_pool(name="sb", bufs=1) as sb,
            tc.tile_pool(name="sb2", bufs=3) as sb2,
        ):
            # ---------------- loads ----------------
            W_sb = sb.tile([128, KC, NPAD], DT, tag="W")
            mT_sb = sb.tile([128, KC, I], DT, tag="mT")
            qT_sb = sb.tile([128, KC, QL], DT, tag="qT")
            eye = sb.tile([128, 128], F32, tag="eye")
            nc.sync.dma_start(mT_sb[:], mT_d[:].rearrange("(k p) n -> p k n", p=128).bitcast(DT))
            nc.sync.dma_start(qT_sb[:], qT_d[:].rearrange("(k p) n -> p k n", p=128).bitcast(DT))
            nc.sync.dma_start(eye[:], eye_d[:])
            Wr = W_d[:].rearrange("(k p) n -> p k n", p=128).bitcast(DT)
            for k in range(KC):
                nc.sync.dma_start(W_sb[:, k, 0:512], Wr[:, k, 0:512])
            for k in range(KC):
                nc.sync.dma_start(W_sb[:, k, 512:768], Wr[:, k, 512:768])
            if with_bias:
                b_sb = sb.tile([1, CD], F32, tag="b")
                nc.sync.dma_start(b_sb[:], b_d[:])
            ones1 = sb.tile([1, 128], F32, tag="ones1")
            nc.vector.memset(ones1[:], 1.0)
            halfv = sb.tile([1, 128], F32, tag="halfv")
            nc.vector.memset(halfv[:], 0.5)
            onesD = sb.tile([128, 1], DT, tag="onesD")
            nc.sync.dma_start(onesD[:], ones_d[:].bitcast(DT))
            epsb = sb.tile([128, 1], F32, tag="epsb")
            nc.vector.memset(epsb[:], EPS)
            onesC = sb.tile([34, 1], DT, tag="onesC")
            nc.sync.dma_start(onesC[:], onesc_d[:].bitcast(DT))

            # ---------------- projections (hat-major) ----------------
            hat_m_r = sb.tile([128, CD + 1], DT, tag="hatmr")  # col 765 zero (even-N pad)
            hat_q32 = sb.tile([QL, CD], F32, tag="hatq32")

            with tc.tile_pool(name="ps1", bufs=1, space="PSUM") as ps1, \
                 tc.tile_pool(name="pstp", bufs=4, space="PSUM") as pstp:
                psA = ps1.tile([128, 512], F32, tag="psA")
                psB = ps1.tile([128, 256], F32, tag="psB")
                for k in range(KC):
                    nc.tensor.matmul(psA[:], mT_sb[:, k, :], W_sb[:, k, 0:512],
                                     start=(k == 0), stop=(k == KC - 1 and not with_bias))
                    nc.tensor.matmul(psB[:], mT_sb[:, k, :], W_sb[:, k, 512:768],
                                     start=(k == 0), stop=(k == KC - 1 and not with_bias))
                if with_bias:
                    nc.tensor.matmul(psA[:], ones1[:], b_sb[:, 0:512], start=False, stop=True)
                    nc.tensor.matmul(psB[:, 0:253], ones1[:], b_sb[:, 512:765],
                                     start=False, stop=True)
                nc.scalar.copy(hat_m_r[:, 0:512], psA[:])
                nc.vector.tensor_copy(hat_m_r[:, 512:765], psB[:, 0:253])
                nc.sync.dma_start(hat_m_r[:, 765:766], zeros_d[0:128, 640:641].bitcast(DT))

                psC = ps1.tile([QL, 512], F32, tag="psC")
                psD = ps1.tile([QL, 256], F32, tag="psD")
                for k in range(KC):
                    nc.tensor.matmul(psC[:], qT_sb[:, k, :], W_sb[:, k, 0:512],
                                     start=(k == 0), stop=(k == KC - 1 and not with_bias))
                    nc.tensor.matmul(psD[:], qT_sb[:, k, :], W_sb[:, k, 512:768],
                                     start=(k == 0), stop=(k == KC - 1 and not with_bias))
                if with_bias:
                    onesq = sb.tile([1, QL], F32, tag="onesq")
                    nc.vector.memset(onesq[:], 1.0)
                    nc.tensor.matmul(psC[:], onesq[:], b_sb[:, 0:512],
                                     start=False, stop=True)
                    nc.tensor.matmul(psD[:, 0:253], onesq[:], b_sb[:, 512:765],
                                     start=False, stop=True)
                # NOTE: bias-for-q path writes b broadcast over q? must be b per column:
                # out[q, n] += 1*b[n] -> lhsT = onesq [1, QL], rhs = b [1, n] OK.
                nc.scalar.copy(hat_q32[:, 0:512], psC[:])
                nc.scalar.copy(hat_q32[:, 512:765], psD[:, 0:253])

                # ---------------- m stats ----------------
                # mum [128, C], xn2 [128, C], inv_xn [128, C]
                hm32 = hat_m_r[:, 0:765].bitcast(F32)
                mum = sb.tile([128, C], F32, tag="mum")
                nc.vector.tensor_reduce(mum[:], hm32.rearrange("p (c d) -> p c d", c=C),
                                        axis=AX, op=ADD)  # holds D*mean
                sqm = sb.tile([128, CD], F32, tag="sqm")
                nc.vector.tensor_tensor(sqm[:], hm32, hm32, op=MUL)
                xn2 = sb.tile([128, C], F32, tag="xn2")
                nc.vector.tensor_reduce(xn2[:], sqm[:].rearrange("p (c d) -> p c d", c=C),
                                        axis=AX, op=ADD)
                # xn2 = sum(hm^2) - D*mum^2 ; inv_xn = 1/sqrt(xn2)
                mum2 = sb.tile([128, C], F32, tag="mum2")
                nc.vector.tensor_tensor(mum2[:], mum[:], mum[:], op=MUL)
                nc.vector.tensor_scalar(mum2[:], mum2[:], 1.0 / D, None, op0=MUL)
                nc.vector.tensor_tensor(xn2[:], xn2[:], mum2[:], op=SUB)
                lxn = sb.tile([128, C], F32, tag="lxn")
                nc.scalar.activation(lxn[:], xn2[:], mybir.ActivationFunctionType.Ln)
                inv_xn = sb.tile([128, C], F32, tag="invxn")
                nc.scalar.activation(inv_xn[:], lxn[:], mybir.ActivationFunctionType.Exp, scale=-0.5)

                # rows: [C, 128] transposes of mum and inv_xn
                tpm = pstp.tile([C, 128], F32, tag="tp")
                nc.tensor.transpose(tpm[:], mum[:], eye[:])
                mumT = sb.tile([C, 128], F32, tag="mumT")
                nc.scalar.copy(mumT[:], tpm[:])
                nmumT = sb.tile([C, 128], F32, tag="nmumT")
                nc.vector.tensor_scalar(nmumT[:], mumT[:], -1.0 / D, None, op0=MUL)
                tpx = pstp.tile([C, 128], F32, tag="tp")
                nc.tensor.transpose(tpx[:], inv_xn[:], eye[:])
                invxnT = sb.tile([C, 128], F32, tag="invxnT")
                nc.scalar.copy(invxnT[:], tpx[:])
                # matmul lhsT needs base_partition 0: stage each row at partition 0
                rowsX = sb.tile([1, C, 128], F32, tag="rowsX")
                for c in range(C):
                    nc.sync.dma_start(rowsX[:, c, :], invxnT[c:c + 1, :])

                # ---------------- transposes: hmT (aug) and tq ----------------
                mTc1 = sb.tile([128, C, 128], DT, tag="mTc1")   # rows d=0..127
                mTc2 = sb.tile([34, C, 128], DT, tag="mTc2")    # rows d=128..152, row32=-mum, rest 0
                tqA = sb.tile([128, C, QL], DT, tag="tqA")
                tqB = sb.tile([34, C, QL], DT, tag="tqB")       # row32 = colsum(tq), rows 25..31,33 zero
                vA = sb.tile([128, C, QL], DT, tag="vA")
                vB = sb.tile([34, C, QL], DT, tag="vB")         # rows 25..33 stay 0
                nc.sync.dma_start(vB[:], zeros_d[0:34, 0:CQ].rearrange("p (c q) -> p c q", c=C).bitcast(DT))
                nc.sync.dma_start(tqB[:], zeros_d[0:34, 0:CQ].rearrange("p (c q) -> p c q", c=C).bitcast(DT))
                nc.sync.dma_start(mTc2[:], zeros_d[0:34, 0:640].rearrange("p (c q) -> p c q", c=C).bitcast(DT))

                for c in range(C):
                    t1 = pstp.tile([128, 128], F32, tag="tp")
                    nc.tensor.transpose(t1[:], hat_m_r[:, D * c:D * c + 128].bitcast(F32), eye[:])
                    (nc.vector.tensor_copy if c % 2 else nc.scalar.copy)(mTc1[:, c, :], t1[:])
                    t2 = pstp.tile([25, 128], F32, tag="tp")
                    nc.tensor.transpose(t2[:], hat_m_r[:, D * c + 128:D * (c + 1)].bitcast(F32), eye[:])
                    (nc.scalar.copy if c % 2 else nc.vector.tensor_copy)(mTc2[0:25, c, :], t2[:])
                    nc.sync.dma_start(mTc2[32:33, c, :], nmumT[c:c + 1, :].bitcast(DT))

                    t3 = pstp.tile([128, QL], F32, tag="tp")
                    nc.tensor.transpose(t3[:], hat_q32[:, D * c:D * c + 128], eye[0:QL, 0:QL])
                    (nc.vector.tensor_copy if c % 2 else nc.scalar.copy)(tqA[:, c, :], t3[:])
                    t4 = pstp.tile([25, QL], F32, tag="tp")
                    nc.tensor.transpose(t4[:], hat_q32[:, D * c + 128:D * (c + 1)], eye[0:QL, 0:QL])
                    (nc.scalar.copy if c % 2 else nc.vector.tensor_copy)(tqB[0:25, c, :], t4[:])

            if dbg:
                nc.sync.dma_start(dbg_d["hatm"][:], hat_m_r[:, 0:765].bitcast(F32))
                nc.sync.dma_start(dbg_d["hatq"][:], hat_q32[:])
                nc.sync.dma_start(dbg_d["tqA0"][:], tqA[:].bitcast(F32).rearrange("p c q -> p (c q)"))
                nc.sync.dma_start(dbg_d["mTc1d"][:], mTc1[:].bitcast(F32).rearrange("p c q -> p (c q)"))
                nc.sync.dma_start(dbg_d["mTc2d"][:], mTc2[:].bitcast(F32).rearrange("p c q -> p (c q)"))
            # ---------------- routing ----------------
            with tc.tile_pool(name="ps2", bufs=1, space="PSUM") as ps2:
                p_t = None     # pearson tile [128, CQ] fp32
                a_t = None     # routing logits [128, CQ] fp32

                def pearson():
                    """colsum -> row32; yn2 via weighted ones-matmul; p = tanh(num*bc)."""
                    tqA32 = tqA[:].bitcast(F32).rearrange("p c q -> p (c q)")
                    sqA = sb2.tile([128, CQ], DT, tag="sqA")
                    nc.gpsimd.tensor_tensor(sqA[:], tqA32, tqA32, op=MUL)
                    colsum = ps2.tile([1, CQ], F32, tag="colsum")
                    nc.tensor.matmul(colsum[:], onesD[:, :], tqA[:].rearrange("p c q -> p (c q)"),
                                     start=True, stop=False)
                    nc.tensor.matmul(colsum[:], onesD[0:26, :], tqB[0:26].rearrange("p c q -> p (c q)"),
                                     start=False, stop=True)
                    # colsum into tqB row 32 (augmented num matmul + yn2 correction)
                    nc.scalar.copy(tqB[32:33, :, :].rearrange("p c q -> p (c q)"), colsum[:])
                    tqB34 = tqB[0:34].bitcast(F32).rearrange("p c q -> p (c q)")
                    sqB = sb2.tile([34, CQ], DT, tag="sqB")
                    nc.gpsimd.tensor_tensor(sqB[:], tqB34, tqB34, op=MUL)
                    # yn2 = 1'sqA + onesC'sqB  (onesC row32 = -1/D weights colsum^2)
                    yn2 = ps2.tile([1, CQ], F32, tag="colsum2")
                    nc.tensor.matmul(yn2[:], onesD[:, :], sqA[:], start=True, stop=False)
                    nc.tensor.matmul(yn2[:], onesC[:, :], sqB[:], start=False, stop=True)
                    lyn = sb2.tile([1, CQ], F32, tag="lyn")
                    nc.scalar.activation(lyn[:], yn2[:], mybir.ActivationFunctionType.Ln)
                    inv_yn = sb2.tile([1, CQ], F32, tag="invyn")
                    nc.scalar.activation(inv_yn[:], lyn[:], mybir.ActivationFunctionType.Exp, scale=-0.5)

                    # num[i, (c,q)]
                    num = ps2.tile([128, C, QL], F32, tag="num")
                    for c in range(C):
                        nc.tensor.matmul(num[:, c, :], mTc1[:, c, :], tqA[:, c, :],
                                         start=True, stop=False)
                        nc.tensor.matmul(num[:, c, :], mTc2[:, c, :], tqB[:, c, :],
                                         start=False, stop=True)
                    # nx[i,(c,q)] = num * inv_xn[i,c]  (early, parallel with yn chain)
                    nx = sb2.tile([128, C, QL], F32, tag="nx")
                    for c in range(C):
                        nc.vector.tensor_scalar(nx[:, c, :], num[:, c, :], inv_xn[:, c:c + 1],
                                                None, op0=MUL)
                    iyb = ps2.tile([128, CQ], F32, tag="bcast")
                    nc.tensor.matmul(iyb[:], ones1[:], inv_yn[:], start=True, stop=True)
                    pp = sb2.tile([128, CQ], F32, tag="pp")
                    nc.vector.tensor_tensor(pp[:], nx[:].rearrange("p c q -> p (c q)"), iyb[:], op=MUL)
                    # tanh(x) = 1 - 2/(1+exp(2x))
                    e2 = sb2.tile([128, CQ], F32, tag="e2")
                    nc.scalar.activation(e2[:], pp[:], mybir.ActivationFunctionType.Exp, scale=2.0)
                    den = sb2.tile([128, CQ], F32, tag="dent")
                    nc.vector.tensor_scalar(den[:], e2[:], 1.0, None, op0=ADD)
                    rr = sb2.tile([128, CQ], F32, tag="rr")
                    nc.vector.reciprocal(rr[:], den[:])
                    p_new = sb2.tile([128, CQ], F32, tag="p")
                    nc.vector.tensor_scalar(p_new[:], rr[:], -2.0, 1.0, op0=MUL, op1=ADD)
                    return p_new

                p_t = pearson()
                if dbg:
                    nc.sync.dma_start(dbg_d["p1"][:], p_t[:])
                    nc.sync.dma_start(dbg_d["tqB0"][:], tqB[:].bitcast(F32).rearrange("p c q -> p (c q)"))

                for it in range(2):
                    dsp = sb2.tile([128, C, QL], DT, tag="dsp")
                    if it == 0:
                        # softmax(0) = 1/C exactly
                        nc.vector.tensor_scalar(dsp[:].rearrange("p c q -> p (c q)"),
                                                p_t[:], 1.0 / C, None, op0=ADD)
                    else:
                        ea = sb2.tile([128, CQ], F32, tag="ea")
                        nc.scalar.activation(ea[:], a_t[:], mybir.ActivationFunctionType.Exp)
                        asum = sb2.tile([128, QL], F32, tag="asum")
                        nc.vector.tensor_reduce(asum[:], ea[:].rearrange("p (c q) -> p q c", c=C),
                                                axis=AX, op=ADD)
                        rs = sb2.tile([128, QL], F32, tag="rs")
                        nc.vector.reciprocal(rs[:], asum[:])
                        dd = sb2.tile([128, C, QL], F32, tag="dd")
                        nc.vector.tensor_tensor(
                            dd[:], ea[:].rearrange("p (c q) -> p c q", c=C),
                            rs[:].rearrange("p (a q) -> p a q", a=1).broadcast_to((128, C, QL)),
                            op=MUL)
                        nc.vector.tensor_tensor(dsp[:].rearrange("p c q -> p (c q)"),
                                                dd[:].rearrange("p c q -> p (c q)"), p_t[:], op=ADD)

                    # hv[d, (c,q)] in two D-chunks
                    hvA = ps2.tile([128, C, QL], F32, tag="hvA")
                    hvB = ps2.tile([26, C, QL], F32, tag="hvB")
                    for c in range(C):
                        nc.tensor.matmul(hvA[:, c, :], hat_m_r[:, D * c:D * c + 128], dsp[:, c, :],
                                         start=True, stop=True)
                        nc.tensor.matmul(hvB[:, c, :], hat_m_r[:, D * c + 128:D * c + 154], dsp[:, c, :],
                                         start=True, stop=True)
                    # squash scale s[(c,q)] = n2/(1+n2)/sqrt(n2+eps)
                    # stage raw hv into the v tiles (scaled-by-s only where needed)
                    vAf = vA[:].bitcast(F32).rearrange("p c q -> p (c q)")
                    vBf = vB[0:25].bitcast(F32).rearrange("p c q -> p (c q)")
                    nc.scalar.copy(vA[:].rearrange("p c q -> p (c q)"), hvA[:].rearrange("p c q -> p (c q)"))
                    nc.vector.tensor_copy(vB[0:25].rearrange("p c q -> p (c q)"), hvB[0:25].rearrange("p c q -> p (c q)"))
                    sqhA = sb2.tile([128, CQ], DT, tag="sqhA")
                    nc.gpsimd.tensor_tensor(sqhA[:], vAf, vAf, op=MUL)
                    sqhB = sb2.tile([25, CQ], DT, tag="sqhB")
                    nc.gpsimd.tensor_tensor(sqhB[:], vBf, vBf, op=MUL)
                    n2 = ps2.tile([1, CQ], F32, tag="colsum2")  # share slot with colsum2
                    nc.tensor.matmul(n2[:], onesD[:, :], sqhA[:], start=True, stop=False)
                    nc.tensor.matmul(n2[:], onesD[0:25, :], sqhB[:], start=False, stop=True)
                    n2p1 = sb2.tile([1, CQ], F32, tag="n2p1")
                    nc.vector.tensor_scalar(n2p1[:], n2[:], 1.0, None, op0=ADD)
                    r1 = sb2.tile([1, CQ], F32, tag="r1")
                    nc.vector.reciprocal(r1[:], n2p1[:])
                    ln2 = sb2.tile([1, CQ], F32, tag="ln2")
                    nc.scalar.activation(ln2[:], n2[:], mybir.ActivationFunctionType.Ln, bias=epsb[0:1, :])
                    r2 = sb2.tile([1, CQ], F32, tag="r2")
                    nc.scalar.activation(r2[:], ln2[:], mybir.ActivationFunctionType.Exp, scale=-0.5)
                    omr = sb2.tile([1, CQ], F32, tag="omr")
                    nc.vector.tensor_scalar(omr[:], r1[:], -1.0, 1.0, op0=MUL, op1=ADD)
                    srow = sb2.tile([1, CQ], F32, tag="srow")
                    nc.vector.tensor_tensor(srow[:], omr[:], r2[:], op=MUL)
                    # broadcast s to all partitions via ones-matmul
                    sB = ps2.tile([128, CQ], F32, tag="bcast")  # share slot with iyb
                    nc.tensor.matmul(sB[:], ones1[:], srow[:], start=True, stop=True)
                    sBh = ps2.tile([128, CQ], F32, tag="num")  # 0.5*s broadcast; reuses num slot
                    nc.tensor.matmul(sBh[:], halfv[:], srow[:], start=True, stop=True)

                    # mdv[i, (c,q)]
                    mdv = ps2.tile([128, C, QL], F32, tag="mdv")
                    for c in range(C):
                        nc.tensor.matmul(mdv[:, c, :], mTc1[:, c, :], vA[:, c, :],
                                         start=True, stop=False)
                        nc.tensor.matmul(mdv[:, c, :], mTc2[:, c, :], vB[:, c, :],
                                         start=False, stop=True)
                    # a += p * s * mdv_raw   (mdv computed on raw hv; s applied here)
                    pm = sb2.tile([128, CQ], F32, tag="pm")
                    nc.vector.tensor_tensor(pm[:], mdv[:].rearrange("p c q -> p (c q)"), p_t[:], op=MUL)
                    pms = sb2.tile([128, CQ], F32, tag="pms")
                    nc.vector.tensor_tensor(pms[:], pm[:], sB[:], op=MUL)
                    if it == 0:
                        a_t = pms
                    else:
                        a_new = sb2.tile([128, CQ], F32, tag="a")
                        nc.vector.tensor_tensor(a_new[:], a_t[:], pms[:], op=ADD)
                        a_t = a_new

                    # tq = 0.5*tq (computed early) + (0.5*s)*hv_raw
                    tqhA = sb2.tile([128, CQ], F32, tag="tqhA")
                    nc.vector.tensor_scalar(tqhA[:], tqA[:].bitcast(F32).rearrange("p c q -> p (c q)"),
                                            0.5, None, op0=MUL)
                    tqhB = sb2.tile([25, CQ], F32, tag="tqhB")
                    nc.vector.tensor_scalar(tqhB[:], tqB[0:25].bitcast(F32).rearrange("p c q -> p (c q)"),
                                            0.5, None, op0=MUL)
                    svA = sb2.tile([128, CQ], F32, tag="svA")
                    nc.vector.tensor_tensor(svA[:], vAf, sBh[:], op=MUL)
                    nc.vector.tensor_tensor(tqA[:].rearrange("p c q -> p (c q)"), tqhA[:], svA[:], op=ADD)
                    svB = sb2.tile([25, CQ], F32, tag="svB")
                    nc.vector.tensor_tensor(svB[:], vBf, sBh[0:25, :], op=MUL)
                    nc.vector.tensor_tensor(tqB[0:25].rearrange("p c q -> p (c q)"), tqhB[:], svB[:], op=ADD)

                    p_t = pearson()
                    if dbg:
                        nc.sync.dma_start(dbg_d["a1" if it == 0 else "a2"][:], a_t[:])
                        nc.sync.dma_start(dbg_d["p2" if it == 0 else "p3"][:], p_t[:])

                # ---------------- final ----------------
                ea = sb2.tile([128, CQ], F32, tag="ea")
                nc.scalar.activation(ea[:], a_t[:], mybir.ActivationFunctionType.Exp)
                asum = sb2.tile([128, QL], F32, tag="asum")
                nc.vector.tensor_reduce(asum[:], ea[:].rearrange("p (c q) -> p q c", c=C),
                                        axis=AX, op=ADD)
                rs = sb2.tile([128, QL], F32, tag="rs")
                nc.vector.reciprocal(rs[:], asum[:])
                dd = sb2.tile([128, C, QL], F32, tag="dd")
                nc.vector.tensor_tensor(
                    dd[:], ea[:].rearrange("p (c q) -> p c q", c=C),
                    rs[:].rearrange("p (a q) -> p a q", a=1).broadcast_to((128, C, QL)), op=MUL)
                dspF = sb2.tile([128, C, QL], DT, tag="dsp")
                nc.vector.tensor_tensor(dspF[:].rearrange("p c q -> p (c q)"),
                                        dd[:].rearrange("p c q -> p (c q)"), p_t[:], op=ADD)

                hvF = sb.tile([QL, CD], F32, tag="hvF")
                for c in range(C):
                    fps = ps2.tile([QL, D + 1], F32, tag=("hvA" if c % 2 == 0 else "mdv"))
                    nc.tensor.matmul(fps[:], dspF[:, c, :], hat_m_r[:, D * c:D * c + 154],
                                     start=True, stop=True)
                    (nc.vector.tensor_copy if c % 2 else nc.scalar.copy)(hvF[:, D * c:D * (c + 1)], fps[:, 0:153])

                n2q = sb2.tile([QL, C], F32, tag="n2q")
                sqf = sb2.tile([QL, CD], F32, tag="sqf")
                nc.vector.tensor_tensor(sqf[:], hvF[:], hvF[:], op=MUL)
                nc.vector.tensor_reduce(n2q[:], sqf[:].rearrange("p (c d) -> p c d", c=C),
                                        axis=AX, op=ADD)
                fp1 = sb2.tile([QL, C], F32, tag="fp1")
                nc.vector.tensor_scalar(fp1[:], n2q[:], 1.0, None, op0=ADD)
                fr1 = sb2.tile([QL, C], F32, tag="fr1")
                nc.vector.reciprocal(fr1[:], fp1[:])
                fln = sb2.tile([QL, C], F32, tag="fln")
                nc.scalar.activation(fln[:], n2q[:], mybir.ActivationFunctionType.Ln, bias=epsb[0:QL, :])
                fr2 = sb2.tile([QL, C], F32, tag="fr2")
                nc.scalar.activation(fr2[:], fln[:], mybir.ActivationFunctionType.Exp, scale=-0.5)
                fs1 = sb2.tile([QL, C], F32, tag="fs1")
                nc.vector.tensor_scalar(fs1[:], fr1[:], -1.0, 1.0, op0=MUL, op1=ADD)
                fs = sb2.tile([QL, C], F32, tag="fs")
                nc.vector.tensor_tensor(fs[:], fs1[:], fr2[:], op=MUL)
                outT = sb.tile([QL, CD], F32, tag="outT")
                nc.vector.tensor_tensor(
                    outT[:].rearrange("p (c d) -> p c d", c=C),
                    hvF[:].rearrange("p (c d) -> p c d", c=C),
                    fs[:].rearrange("p (c a) -> p c a", a=1).broadcast_to((QL, C, D)), op=MUL)
                nc.sync.dma_start(out_d[:], outT[:])

    # All activations use only {Ln, Exp, Copy}, which live together in act
    # func set 6 (natural_log_exp_and_others). The default solver alternates
    # sets 0/5, inserting ~15 table reloads (~1.3us each); one load suffices.
    def _single_act_table_load():
        inst = mybir.InstLoadActFuncSet(
            name=nc.get_next_instruction_name(), ins=[], outs=[],
            act_func_set_id=6,
        )
        inst.engine = mybir.EngineType.Activation
        nc.register_instruction(inst)
        for blk in nc.main_func.blocks:
            for idx, bi in enumerate(blk.instructions):
                if isinstance(bi, mybir.InstActivation):
                    blk.instructions.insert(idx, inst)
                    return
        raise AssertionError("no activation found")

    nc.insert_act_table_loads = _single_act_table_load
    nc.compile()
    return nc


_CACHE = {}
LAST_EXEC_NS = None
LAST_RESULTS = None


def kernel(m, q, W, b):
    m = np.asarray(m, dtype=np.float32)
    q = np.asarray(q, dtype=np.float32)
    W = np.asarray(W, dtype=np.float32)
    b = np.asarray(b, dtype=np.float32)
    assert m.shape == (I, K) and q.shape == (NCORES * QL, K) and W.shape == (K, CD)

    with_bias = bool(np.any(b))
    dbg = bool(int(os.environ.get("KERNEL_DBG", "0")))
    key = ("v1", with_bias, str(DT), dbg)
    if key not in _CACHE:
        _CACHE[key] = build(with_bias, dbg)
    nc = _CACHE[key]

    Wp = np.zeros((K, NPAD), dtype=np.float32)
    Wp[:, :CD] = W
    mT = np.ascontiguousarray(m.T)
    eye = np.eye(128, dtype=np.float32)
    b2 = b.reshape(1, CD)

    onesv = np.ones((128, 1), dtype=np.float32)
    zerosv = np.zeros((128, 644), dtype=np.float32)
    onescv = np.zeros((34, 1), dtype=np.float32)
    onescv[0:25] = 1.0
    onescv[32] = -1.0 / D
    in_maps = []
    for i in range(NCORES):
        qT = np.ascontiguousarray(q[QL * i:QL * (i + 1)].T)
        in_maps.append({"mT": mT, "qT": qT, "Wp": Wp, "b": b2, "eye": eye,
                        "onesv": onesv, "zerosv": zerosv, "onescv": onescv})

    res = run_bass_kernel_spmd(nc, in_maps, list(range(NCORES)))
    global LAST_EXEC_NS, LAST_RESULTS
    LAST_EXEC_NS = res.exec_time_ns
    LAST_RESULTS = res.results
    out = np.concatenate([res.results[i]["out"] for i in range(NCORES)], axis=0)
    return out.astype(np.float32)


if __name__ == "__main__":
    rng = np.random.default_rng(0)
    m = rng.standard_normal((I, K)).astype(np.float32)
    q = rng.standard_normal((NCORES * QL, K)).astype(np.float32)
    W = (rng.standard_normal((K, CD)) * 0.02).astype(np.float32)
    b = np.zeros((CD,), dtype=np.float32)
    out = kernel(m=m, q=q, W=W, b=b)
    print("out", out.shape, out.dtype, np.abs(out).mean())



# revision 23
# speedup vs baseline: 1.6972x; 1.6972x over previous
"""DMR induction routing kernel for Trainium2 (Bass/Tile), 8-core data-parallel.

Problem: nn_DMRInduction. Full inputs:
  m [128, 768], q [256, 768], W [768, 765], b [765] -> out [256, 765] fp32.

Sharding: Q=256 split 8 ways (32 queries/core); m, W, b replicated.

Gram-matrix reformulation: instead of carrying the routing query state
tmp_q as a [D, C*Q] tensor through each iteration, precompute per-class
Gram matrices of the (projected) memory capsules
    Gc[c][j,i]   = xm_j . xm_i   (xm = hat_m centered over d; equals
                                  m_j . xm_i since sum_d xm = 0)
    Graw[c][j,i] = m_j . m_i     (= Gc + mum_j . mum_i / D, rank-1)
and maintain only
    num [I, C*Q]  = xm_i . tq~    (pearson numerator, one-sided centering)
    yn2 [1, C*Q]  = ||tq~ - mean||^2
with tq~ = 2^k * tq (pearson is scale-invariant; 2^k absorbs the 0.5 mixing).
Per iteration, with dsp = softmax(a)+p (hv = M @ dsp is never materialized):
    nh_raw = Graw @ dsp  (= m_i . hv)     nh_c = Gc @ dsp  (= xm_i . hv)
    n2    = <dsp, nh_raw>  = ||hv||^2          (per column, ones-matmul)
    cross = <dsp, num>     = hv_c . tq~_c      (exact, no correction)
    hvn2  = <dsp, nh_c>    = ||hv_c||^2        (exact, no correction)
    s = sqrt(n2+eps)/(1+n2);  sc = 2^it * s
    a   += p * s * nh_raw
    num += sc * nh_c
    yn2 += 2*sc*cross + sc^2*hvn2
    p = tanh(num * inv_xn * rsqrt(yn2))
All loop matmuls are [128x128] x [128x32] per class; the rest is row math.
tq itself is materialized once (tq0 = W^T q in [d,(c,q)] layout) for the
initial pearson only.

NOTE: PSUM matmul accumulation groups must be CONTIGUOUS in program order
(interleaving accumulation groups corrupts results -- found empirically;
the psA/psB per-chunk interleave works because they sit in separate banks).
"""
import os
import sys

for _p in ("/opt/trn_rl_repo", "/root/.axon_site/_ro/trn_rl_repo"):
    if os.path.isdir(_p) and _p not in sys.path:
        sys.path.insert(0, _p)

import numpy as np
import concourse.bass as bass
import concourse.bacc as bacc
import concourse.mybir as mybir
import concourse.tile as tile
from concourse.bass_utils import run_bass_kernel_spmd

F32 = mybir.dt.float32
BF16 = mybir.dt.bfloat16

NCORES = 8
I = 128         # memory capsules
C = 5           # capsule classes
D = 153         # dim per capsule
CD = C * D      # 765
K = 768         # input dim
KC = K // 128   # 6 contraction chunks
QL = 32         # queries per core
CQ = C * QL     # 160
NPAD = 768      # W padded to 768 cols
EPS = 1e-8
NWARM = int(os.environ.get("KERNEL_NWARM", "26"))
AX = mybir.AxisListType.X
MUL = mybir.AluOpType.mult
ADD = mybir.AluOpType.add
LN = mybir.ActivationFunctionType.Ln
EXP = mybir.ActivationFunctionType.Exp
SQF = mybir.ActivationFunctionType.Square

USE_BF16 = os.environ.get("KERNEL_F32", "0") != "1"
DT = BF16 if USE_BF16 else F32
DT_IO = BF16 if USE_BF16 else F32


def build(with_bias: bool, dbg: bool = False):
    nc = bacc.Bacc("TRN2", target_bir_lowering=False, debug=False)

    mT_d = nc.dram_tensor("mT", [K, I], DT_IO, kind="ExternalInput")
    qT_d = nc.dram_tensor("qT", [K, QL], DT_IO, kind="ExternalInput")
    W_d = nc.dram_tensor("Wp", [K, NPAD], DT_IO, kind="ExternalInput")
    Ws_d = nc.dram_tensor("Wsum", [K, 8], DT_IO, kind="ExternalInput")
    eye_d = nc.dram_tensor("eye", [128, 128], DT_IO, kind="ExternalInput")
    out_d = nc.dram_tensor("out", [QL, CD], F32, kind="ExternalOutput")
    if with_bias:
        b_d = nc.dram_tensor("b", [1, CD], DT_IO, kind="ExternalInput")
        bs_d = nc.dram_tensor("bsum", [1, 8], DT_IO, kind="ExternalInput")
    dbg_d = {}
    if dbg:
        for nm, shp, dt_ in [
            ("hatm", [128, CD], DT), ("tqA0", [128, CQ], DT), ("tqB0", [34, CQ], DT),
            ("num0", [128, CQ], F32), ("yn20", [1, CQ], F32), ("p1", [128, CQ], F32),
            ("Gr0", [128, 128], DT), ("Gc0", [128, 128], DT),
            ("n2_0", [1, CQ], F32), ("s_0", [1, CQ], F32),
            ("num_1", [128, CQ], F32), ("yn2_1", [1, CQ], F32),
            ("a_1", [128, CQ], F32), ("p2", [128, CQ], F32),
            ("a_2", [128, CQ], F32), ("p3", [128, CQ], F32),
            ("dspF", [128, CQ], DT), ("hvF", [QL, CD], F32),
            ("dsp0", [128, CQ], DT), ("nh0", [128, CQ], F32), ("t2_0", [128, CQ], DT),
        ]:
            dbg_d[nm] = nc.dram_tensor("dbg_" + nm, shp, dt_, kind="ExternalOutput")

    with tile.TileContext(nc) as tc:
        with (
            tc.tile_pool(name="sb", bufs=1) as sb,
            tc.tile_pool(name="sb2", bufs=2) as sb2,
        ):
            # ---------------- constants (memset, no DMA) ----------------
            ones1 = sb.tile([1, 128], DT, tag="ones1")      # bcast mm lhsT
            nc.vector.memset(ones1[:], 1.0)
            onesD = sb.tile([128, 1], DT, tag="onesD")      # col-reduce mm lhsT
            nc.vector.memset(onesD[:], 1.0)
            onesC = sb.tile([34, 1], DT, tag="onesC")       # aug col-reduce lhsT
            nc.vector.memset(onesC[:], 0.0)
            nc.vector.memset(onesC[0:25, :], 1.0)
            nc.vector.memset(onesC[32:33, :], -1.0 / D)
            onesQ = sb.tile([128, QL], F32, tag="onesQ")    # for row expansion
            nc.gpsimd.memset(onesQ[:], 1.0)
            epsb = sb.tile([128, 1], F32, tag="epsb")
            nc.gpsimd.memset(epsb[:], EPS)
            if with_bias:
                onesQr = sb.tile([1, QL], DT, tag="onesQr")
                nc.vector.memset(onesQr[:], 1.0)

            # ---------------- DMA loads ----------------
            W_sb = sb.tile([128, KC, NPAD], DT, tag="W")
            mT_sb = sb.tile([128, KC, I], DT, tag="mT")
            qT_sb = sb.tile([128, KC, QL], DT, tag="qT")
            Ws_sb = sb.tile([128, KC, 8], DT, tag="Ws")
            eye = sb.tile([128, 128], DT, tag="eye")
            warmS = sb.tile([128, 128], DT, tag="warmS")
            nc.vector.memset(warmS[:], 0.0)
            Wr = W_d[:].rearrange("(k p) n -> p k n", p=128).bitcast(DT)
            nc.sync.dma_start(W_sb[:, 0, :], Wr[:, 0, :])
            nc.sync.dma_start(mT_sb[:], mT_d[:].rearrange("(k p) n -> p k n", p=128).bitcast(DT))
            nc.sync.dma_start(qT_sb[:], qT_d[:].rearrange("(k p) n -> p k n", p=128).bitcast(DT))
            nc.sync.dma_start(Ws_sb[:], Ws_d[:].rearrange("(k p) n -> p k n", p=128).bitcast(DT))
            for k in range(1, KC):
                nc.sync.dma_start(W_sb[:, k, :], Wr[:, k, :])
            nc.scalar.dma_start(eye[:], eye_d[:].bitcast(DT))
            if with_bias:
                b_sb = sb.tile([1, CD], DT, tag="b")
                nc.scalar.dma_start(b_sb[:], b_d[:].bitcast(DT))
                bs_sb = sb.tile([1, 8], DT, tag="bs")
                nc.scalar.dma_start(bs_sb[:], bs_d[:].bitcast(DT))

            # ---------------- projections ----------------
            with tc.tile_pool(name="ps0", bufs=1, space="PSUM") as ps0:
                psA = ps0.tile([128, 512], F32, tag="psA")      # bank 0
                psB = ps0.tile([128, 512], F32, tag="psB")      # bank 1 (256 used)
                tqA_ps = ps0.tile([128, 512], F32, tag="tqAp")  # bank 2 (160 used)
                tqB_ps = ps0.tile([34, 512], F32, tag="tqBp")   # bank 3

                # PE warm-up: back-to-back matmuls on a memset tile ramp
                # the PE clock during the DMA wait (full speed ~3us busy).
                wt = ps0.tile([128, 128], F32, tag="warm")  # bank 4
                for w in range(NWARM):
                    nc.tensor.matmul(wt[:], warmS[:], warmS[:], start=True, stop=True)

                # hat_m accumulation (psA/psB interleave: separate banks)
                for k in range(KC):
                    st, sp = (k == 0), (k == KC - 1 and not with_bias)
                    nc.tensor.matmul(psA[:], mT_sb[:, k, :], W_sb[:, k, 0:512],
                                     start=st, stop=sp)
                    nc.tensor.matmul(psB[:, 0:256], mT_sb[:, k, :], W_sb[:, k, 512:768],
                                     start=st, stop=sp)
                if with_bias:
                    nc.tensor.matmul(psA[:], ones1[0:1, :], b_sb[:, 0:512],
                                     start=False, stop=True)
                    nc.tensor.matmul(psB[:, 0:253], ones1[0:1, :], b_sb[:, 512:765],
                                     start=False, stop=True)
                # tq: each accumulation group contiguous
                tqA3 = tqA_ps[:, 0:CQ].rearrange("p (c q) -> p c q", c=C)
                tqB3 = tqB_ps[:, 0:CQ].rearrange("p (c q) -> p c q", c=C)
                for c in range(C):
                    for k in range(KC):
                        nc.tensor.matmul(tqA3[:, c, :], W_sb[:, k, D * c:D * c + 128],
                                         qT_sb[:, k, :], start=(k == 0),
                                         stop=(k == KC - 1 and not with_bias))
                    if with_bias:
                        nc.tensor.matmul(tqA3[:, c, :], b_sb[:, D * c:D * c + 128],
                                         onesQr[:], start=False, stop=True)
                for c in range(C):
                    for k in range(KC):
                        nc.tensor.matmul(tqB3[0:25, c, :], W_sb[:, k, D * c + 128:D * c + 153],
                                         qT_sb[:, k, :], start=(k == 0),
                                         stop=(k == KC - 1 and not with_bias))
                    if with_bias:
                        nc.tensor.matmul(tqB3[0:25, c, :], b_sb[:, D * c + 128:D * c + 153],
                                         onesQr[:], start=False, stop=True)
                for c in range(C):
                    for k in range(KC):
                        nc.tensor.matmul(tqB3[32:33, c, :], Ws_sb[:, k, c:c + 1],
                                         qT_sb[:, k, :], start=(k == 0),
                                         stop=(k == KC - 1 and not with_bias))
                    if with_bias:
                        nc.tensor.matmul(tqB3[32:33, c, :], bs_sb[:, c:c + 1],
                                         onesQr[:], start=False, stop=True)

                # psum -> sbuf
                hat_mB = sb.tile([128, 766], DT, tag="hatm")
                nc.scalar.copy(hat_mB[:, 0:512], psA[:])
                nc.vector.tensor_copy(hat_mB[:, 512:765], psB[:, 0:253])
                nc.vector.memset(hat_mB[:, 765:766], 0.0)
                hmf = hat_mB[:, 0:765]

                # tq psum -> sbuf (DVE first: tq groups end before hat_m use)
                tqA = sb.tile([128, C, QL], DT, tag="tqA")
                nc.vector.tensor_copy(tqA[:].rearrange("p c q -> p (c q)"),
                                      tqA_ps[:, 0:CQ])
                tqB = sb.tile([34, C, QL], DT, tag="tqB")
                nc.gpsimd.memset(tqB[:].rearrange("p c q -> p (c q)"), 0.0)
                nc.scalar.copy(tqB[0:25].rearrange("p c q -> p (c q)"),
                               tqB_ps[0:25, 0:CQ])
                nc.vector.tensor_copy(tqB[32:33].rearrange("p c q -> p (c q)"),
                                      tqB_ps[32:33, 0:CQ])

                # stats: mum -> hat_mC first (gates transposes); xn2 after
                mum = sb.tile([128, C], F32, tag="mum")     # sum_d hat_m (= D*mean)
                nc.vector.tensor_reduce(mum[:], hmf.rearrange("p (c d) -> p c d", c=C),
                                        axis=AX, op=ADD)
                sqm = sb.tile([128, CD], DT, tag="sqm")
                nc.scalar.activation(sqm[:], hmf, SQF)
                nmu = sb.tile([128, C], F32, tag="nmu")     # -mean
                nc.vector.tensor_scalar(nmu[:], mum[:], -1.0 / D, None, op0=MUL)
                hat_mC = sb.tile([128, CD], DT, tag="hatmC")
                for c in range(C):
                    nc.vector.tensor_scalar(hat_mC[:, D * c:D * (c + 1)],
                                            hmf[:, D * c:D * (c + 1)],
                                            nmu[:, c:c + 1], None, op0=ADD)
                mumB = sb.tile([128, 8], DT, tag="mumB")
                nc.vector.tensor_copy(mumB[:, 0:C], mum[:])
                nc.vector.memset(mumB[:, C:8], 0.0)
                xn2r = sb.tile([128, C], F32, tag="xn2r")
                nc.vector.tensor_reduce(xn2r[:], sqm[:].rearrange("p (c d) -> p c d", c=C),
                                        axis=AX, op=ADD)
                mum2 = sb.tile([128, C], F32, tag="mum2")
                nc.vector.tensor_tensor(mum2[:], mum[:], mum[:], op=MUL)
                xn2 = sb.tile([128, C], F32, tag="xn2")
                nc.vector.scalar_tensor_tensor(xn2[:], mum2[:], -1.0 / D, xn2r[:],
                                               op0=MUL, op1=ADD)
                lxn = sb.tile([128, C], F32, tag="lxn")
                nc.scalar.activation(lxn[:], xn2[:], LN)
                inv_xn = sb.tile([128, C], F32, tag="invxn")
                nc.scalar.activation(inv_xn[:], lxn[:], EXP, scale=-0.5)
                ixq = sb.tile([128, C, QL], F32, tag="ixq")
                for c in range(C):
                    nc.vector.tensor_scalar(ixq[:, c, :], onesQ[:], inv_xn[:, c:c + 1],
                                            None, op0=MUL)

            # ---------------- transposes: centered mTc + mum rows ------------
            mTc1 = sb.tile([128, C, 128], DT, tag="mTc1")   # xm, d rows 0..127
            mTc2 = sb.tile([25, C, 128], DT, tag="mTc2")    # xm, d rows 128..152
            with tc.tile_pool(name="pstp", bufs=3, space="PSUM") as pstp:
                for c in range(C):
                    t1 = pstp.tile([128, 128], DT, tag="tp")
                    nc.tensor.transpose(t1[:], hat_mC[:, D * c:D * c + 128], eye[:])
                    (nc.vector.tensor_copy if c % 2 else nc.scalar.copy)(mTc1[:, c, :], t1[:])
                    t2 = pstp.tile([25, 128], DT, tag="tp2")
                    nc.tensor.transpose(t2[:], hat_mC[:, D * c + 128:D * (c + 1)], eye[:])
                    (nc.scalar.copy if c % 2 else nc.vector.tensor_copy)(mTc2[:, c, :], t2[:])
                # mum rows at partition 0 via column transposes (no DMA)
                smT = sb.tile([1, C, 128], DT, tag="smT")
                for c in range(C):
                    tpm = pstp.tile([1, 128], DT, tag="tpm", bufs=1)
                    nc.tensor.transpose(tpm[:], mumB[:, c:c + 1], eye[:])
                    (nc.vector.tensor_copy if c % 2 else nc.scalar.copy)(
                        smT[:, c, :], tpm[:])
            smTD = sb.tile([1, C, 128], DT, tag="smTD")
            nc.vector.tensor_scalar(smTD[:].rearrange("p c n -> p (c n)"),
                                    smT[:].rearrange("p c n -> p (c n)"),
                                    1.0 / D, None, op0=MUL)

            # ---------------- initial pearson + G build ----------------
            with tc.tile_pool(name="ps2", bufs=1, space="PSUM") as ps2, \
                 tc.tile_pool(name="ps3", bufs=1, space="PSUM") as ps3:
                p3pack = ps3.tile([128, 3, CQ], F32, tag="p3pack")
                num0_ps = p3pack[:, 0, :].rearrange("p (c q) -> p c q", c=C)
                yn0_ps = p3pack[0:1, 1, :]
                for c in range(C):
                    nc.tensor.matmul(num0_ps[:, c, :], mTc1[:, c, :], tqA[:, c, :],
                                     start=True, stop=False)
                    nc.tensor.matmul(num0_ps[:, c, :], mTc2[:, c, :], tqB[0:25, c, :],
                                     start=False, stop=True)
                sqA = sb2.tile([128, CQ], DT, tag="sqA")
                nc.gpsimd.tensor_tensor(sqA[:], tqA[:].rearrange("p c q -> p (c q)"),
                                        tqA[:].rearrange("p c q -> p (c q)"), op=MUL)
                sqB = sb2.tile([34, CQ], DT, tag="sqB")
                nc.vector.tensor_tensor(sqB[:], tqB[:].rearrange("p c q -> p (c q)"),
                                        tqB[:].rearrange("p c q -> p (c q)"), op=MUL)
                nc.tensor.matmul(yn0_ps, onesD[:], sqA[:], start=True, stop=False)
                nc.tensor.matmul(yn0_ps, onesC[:], sqB[:], start=False, stop=True)

                # G build (contiguous groups; copies emitted after p1 acts)
                Gr_sb = sb.tile([128, C, 128], DT, tag="Gr")
                Gc_sb = sb.tile([128, C, 128], DT, tag="Gc")
                Gr_ps = ps3.tile([128, C, 128], F32, tag="Gp")
                for c in range(C):
                    nc.tensor.matmul(Gr_ps[:, c, :], mTc1[:, c, :], mTc1[:, c, :],
                                     start=True, stop=False)
                    nc.tensor.matmul(Gr_ps[:, c, :], mTc2[:, c, :], mTc2[:, c, :],
                                     start=False, stop=False)
                    nc.tensor.matmul(Gr_ps[:, c, :], smT[:, c, :], smTD[:, c, :],
                                     start=False, stop=True)

                def pchain(num_ap, yn2_ap, iyb, tag):
                    """num, yn2 -> r = 1/(1+exp(2*arg)); p = 1-2r = tanh(arg)."""
                    lyn = sb2.tile([1, CQ], F32, tag="lyn")
                    nc.scalar.activation(lyn[:], yn2_ap, LN)
                    iyn = sb2.tile([1, CQ], DT, tag="iyn")
                    nc.scalar.activation(iyn[:], lyn[:], EXP, scale=-0.5)
                    nc.tensor.matmul(iyb, ones1[:], iyn[:], start=True, stop=True)
                    px = sb2.tile([128, CQ], F32, tag="px")
                    nc.vector.tensor_tensor(px[:], num_ap,
                                            ixq[:].rearrange("p c q -> p (c q)"), op=MUL)
                    pp = sb2.tile([128, CQ], F32, tag="pp")
                    nc.vector.tensor_tensor(pp[:], px[:], iyb, op=MUL)
                    e2 = sb2.tile([128, CQ], F32, tag="e2")
                    nc.scalar.activation(e2[:], pp[:], EXP, scale=2.0)
                    den = sb2.tile([128, CQ], F32, tag="den")
                    nc.vector.tensor_scalar(den[:], e2[:], 1.0, None, op0=ADD)
                    r = sb2.tile([128, CQ], F32, tag="r")
                    nc.vector.reciprocal(r[:], den[:])
                    return r

                bc0 = ps2.tile([128, 3, CQ], F32, tag="bc")
                r_t = pchain(p3pack[:, 0, :], yn0_ps, bc0[:, 0, :], "p1")
                Gc_ps = ps3.tile([128, C, 128], F32, tag="Gcp")
                for c in range(C):
                    nc.tensor.matmul(Gc_ps[:, c, :], mTc1[:, c, :], mTc1[:, c, :],
                                     start=True, stop=False)
                    nc.tensor.matmul(Gc_ps[:, c, :], mTc2[:, c, :], mTc2[:, c, :],
                                     start=False, stop=True)
                # G copies on Act, queued after p1's activations
                nc.scalar.copy(Gr_sb[:].rearrange("p c n -> p (c n)"),
                               Gr_ps[:].rearrange("p c n -> p (c n)"))
                nc.scalar.copy(Gc_sb[:].rearrange("p c n -> p (c n)"),
                               Gc_ps[:].rearrange("p c n -> p (c n)"))

                if dbg:
                    nc.sync.dma_start(dbg_d["hatm"][:], hat_mB[:, 0:765])
                    nc.sync.dma_start(dbg_d["tqA0"][:], tqA[:].rearrange("p c q -> p (c q)"))
                    nc.sync.dma_start(dbg_d["tqB0"][:], tqB[:].rearrange("p c q -> p (c q)"))
                    nc.sync.dma_start(dbg_d["Gr0"][:], Gr_sb[:, 0, :])
                    nc.sync.dma_start(dbg_d["Gc0"][:], Gc_sb[:, 0, :])
                    p1d = sb2.tile([128, CQ], F32, tag="pdbg")
                    nc.vector.tensor_scalar(p1d[:], r_t[:], -2.0, 1.0, op0=MUL, op1=ADD)
                    nc.sync.dma_start(dbg_d["p1"][:], p1d[:])
                    n0d = sb2.tile([128, CQ], F32, tag="n0dbg")
                    nc.vector.tensor_copy(n0d[:], p3pack[:, 0, :])
                    nc.sync.dma_start(dbg_d["num0"][:], n0d[:])
                    y0d = sb2.tile([1, CQ], F32, tag="y0dbg")
                    nc.vector.tensor_copy(y0d[:], yn0_ps)
                    nc.sync.dma_start(dbg_d["yn20"][:], y0d[:])

                num_ap = p3pack[:, 0, :]
                yn2_ap = yn0_ps
                a_t = None
                dd1 = None

                for it in range(2):
                    # --- head: dsp, p ---
                    dsp = sb2.tile([128, C, QL], DT, tag="dsp")
                    dspf = dsp[:].rearrange("p c q -> p (c q)")
                    if it == 0:
                        nc.vector.tensor_scalar(dspf, r_t[:], -2.0, 1.0 / C + 1.0,
                                                op0=MUL, op1=ADD)
                    else:
                        nc.vector.scalar_tensor_tensor(dspf, r_t[:], -2.0, dd1[:],
                                                       op0=MUL, op1=ADD)
                    p_t = sb2.tile([128, CQ], F32, tag="p")
                    nc.vector.tensor_scalar(p_t[:], r_t[:], -2.0, 1.0, op0=MUL, op1=ADD)

                    # --- PE: nh matmuls ---
                    nh_ps = ps2.tile([128, 2, C, QL], F32, tag="nh")
                    bc = ps2.tile([128, 3, CQ], F32, tag="bc")
                    for c in range(C):
                        nc.tensor.matmul(nh_ps[:, 0, c, :], Gr_sb[:, c, :], dsp[:, c, :],
                                         start=True, stop=True)
                    for c in range(C):
                        nc.tensor.matmul(nh_ps[:, 1, c, :], Gc_sb[:, c, :], dsp[:, c, :],
                                         start=True, stop=True)
                    nhrf = nh_ps[:, 0, :, :].rearrange("p c q -> p (c q)")
                    nhcf = nh_ps[:, 1, :, :].rearrange("p c q -> p (c q)")

                    # --- DVE: products ---
                    t2 = sb2.tile([128, CQ], DT, tag="t2")
                    nc.vector.tensor_tensor(t2[:], dspf, nhrf, op=MUL)
                    t1 = sb2.tile([128, CQ], DT, tag="t1")
                    nc.vector.tensor_tensor(t1[:], dspf, num_ap, op=MUL)
                    t4 = sb2.tile([128, CQ], DT, tag="t4")
                    nc.vector.tensor_tensor(t4[:], dspf, nhcf, op=MUL)
                    pm = sb2.tile([128, CQ], F32, tag="pm")
                    nc.vector.tensor_tensor(pm[:], p_t[:], nhrf, op=MUL)

                    n2_ps = p3pack[0:1, 2, :]
                    nc.tensor.matmul(n2_ps, onesD[:], t2[:], start=True, stop=True)
                    crhv = ps2.tile([1, 2, CQ], F32, tag="crhv")
                    nc.tensor.matmul(crhv[:, 0, :], onesD[:], t1[:], start=True, stop=True)
                    nc.tensor.matmul(crhv[:, 1, :], onesD[:], t4[:], start=True, stop=True)
                    n2_ap, cr_ap, hv2_ap = n2_ps, crhv[:, 0, :], crhv[:, 1, :]

                    # --- squash rows; on-path: ln2 -> sqn -> c2s -> yn2n ---
                    ln2 = sb2.tile([1, CQ], F32, tag="ln2")
                    nc.scalar.activation(ln2[:], n2_ap, LN, bias=epsb[0:1, :])
                    sqn = sb2.tile([1, CQ], F32, tag="sqn")
                    nc.scalar.activation(sqn[:], ln2[:], EXP, scale=0.5)
                    n2p1 = sb2.tile([1, CQ], F32, tag="n2p1")
                    nc.vector.tensor_scalar(n2p1[:], n2_ap, 1.0, None, op0=ADD)
                    r1 = sb2.tile([1, CQ], F32, tag="r1")
                    nc.vector.reciprocal(r1[:], n2p1[:])
                    u1 = sb2.tile([1, CQ], F32, tag="u1")       # r1^2
                    nc.vector.tensor_tensor(u1[:], r1[:], r1[:], op=MUL)
                    cr2x = sb2.tile([1, CQ], F32, tag="cr2x")   # 2*2^it*cross
                    nc.vector.tensor_scalar(cr2x[:], cr_ap, float(2.0 * 2.0 ** it),
                                            None, op0=MUL)
                    crr1 = sb2.tile([1, CQ], F32, tag="crr1")
                    nc.vector.tensor_tensor(crr1[:], cr2x[:], r1[:], op=MUL)
                    s2r = sb2.tile([1, CQ], F32, tag="s2r")     # s^2 = (n2+eps)*r1^2
                    nc.vector.tensor_tensor(s2r[:], n2_ap, u1[:], op=MUL)
                    v1 = sb2.tile([1, CQ], F32, tag="v1")       # s^2*hvn2
                    nc.vector.tensor_tensor(v1[:], s2r[:], hv2_ap, op=MUL)
                    yn2a = sb2.tile([1, CQ], F32, tag="yn2a")   # yn2 + 4^it*s^2*hvn2
                    nc.vector.scalar_tensor_tensor(yn2a[:], v1[:], float(4.0 ** it),
                                                   yn2_ap, op0=MUL, op1=ADD)
                    c2s = sb2.tile([1, CQ], F32, tag="c2s")     # 2*sc*cross
                    nc.vector.tensor_tensor(c2s[:], crr1[:], sqn[:], op=MUL)
                    yn2n = sb2.tile([1, CQ], F32, tag="yn2n")
                    nc.vector.tensor_tensor(yn2n[:], yn2a[:], c2s[:], op=ADD)
                    srow = sb2.tile([1, CQ], DT, tag="srow")
                    nc.vector.tensor_tensor(srow[:], r1[:], sqn[:], op=MUL)
                    if it == 0:
                        sra = srow
                    else:
                        sra = sb2.tile([1, CQ], DT, tag="sra")
                        nc.vector.tensor_scalar(sra[:], srow[:], float(2.0 ** it),
                                                None, op0=MUL)
                    sBt = bc[:, 1, :]
                    nc.tensor.matmul(sBt, ones1[:], srow[:], start=True, stop=True)
                    if it == 0:
                        sBa = sBt
                    else:
                        sBa = bc[:, 2, :]
                        nc.tensor.matmul(sBa, ones1[:], sra[:], start=True, stop=True)

                    # num update (sBa via SBUF: TT reads at most one PSUM input)
                    sBa_sb = sb2.tile([128, CQ], F32, tag="sBas")
                    nc.scalar.copy(sBa_sb[:], sBa)
                    t3 = sb2.tile([128, CQ], F32, tag="t3")
                    nc.vector.tensor_tensor(t3[:], sBa_sb[:], nhcf, op=MUL)
                    numn = sb2.tile([128, CQ], F32, tag="numn")
                    nc.vector.tensor_tensor(numn[:], num_ap, t3[:], op=ADD)

                    # a update (t6 on DVE: sBt is PSUM)
                    t6 = sb2.tile([128, CQ], F32, tag="t6")
                    nc.vector.tensor_tensor(t6[:], sBt, pm[:], op=MUL)
                    if it == 0:
                        a_new = t6
                    else:
                        a_new = sb2.tile([128, CQ], F32, tag="a")
                        nc.gpsimd.tensor_tensor(a_new[:], a_t[:], t6[:], op=ADD)
                    a_t = a_new
                    ea = sb2.tile([128, CQ], F32, tag="ea")
                    nc.scalar.activation(ea[:], a_t[:], EXP)

                    num_ap = numn[:]
                    yn2_ap = yn2n[:]
                    r_t = pchain(num_ap, yn2_ap, bc[:, 0, :], f"p{it + 2}")

                    # softmax tail (DVE parts queued after pchain's DVE ops)
                    asum = sb2.tile([128, QL], F32, tag="asum")
                    nc.vector.tensor_reduce(asum[:], ea[:].rearrange("p (c q) -> p q c", c=C),
                                            axis=AX, op=ADD)
                    rs = sb2.tile([128, QL], F32, tag="rs")
                    nc.vector.reciprocal(rs[:], asum[:])
                    dd = sb2.tile([128, C, QL], F32, tag="dd")
                    nc.gpsimd.tensor_tensor(
                        dd[:], ea[:].rearrange("p (c q) -> p c q", c=C),
                        rs[:].rearrange("p (a q) -> p a q", a=1).broadcast_to((128, C, QL)),
                        op=MUL)
                    dd1 = sb2.tile([128, CQ], F32, tag="dd1")
                    nc.gpsimd.tensor_scalar(dd1[:], dd[:].rearrange("p c q -> p (c q)"),
                                            1.0, None, op0=ADD)

                    if dbg:
                        nc.sync.dma_start(dbg_d[f"a_{it + 1}"][:], a_t[:])
                        pd = sb2.tile([128, CQ], F32, tag="pdbg")
                        nc.vector.tensor_scalar(pd[:], r_t[:], -2.0, 1.0, op0=MUL, op1=ADD)
                        nc.sync.dma_start(dbg_d[f"p{it + 2}"][:], pd[:])
                        if it == 0:
                            nc.sync.dma_start(dbg_d["dsp0"][:], dspf)
                            nhd = sb2.tile([128, CQ], F32, tag="n0dbg")
                            nc.vector.tensor_copy(nhd[:], nhrf)
                            nc.sync.dma_start(dbg_d["nh0"][:], nhd[:])
                            nc.sync.dma_start(dbg_d["t2_0"][:], t2[:])
                            n2d = sb2.tile([1, CQ], F32, tag="y0dbg")
                            nc.vector.tensor_copy(n2d[:], n2_ap)
                            nc.sync.dma_start(dbg_d["n2_0"][:], n2d[:])
                            sd = sb2.tile([1, CQ], F32, tag="sdbg")
                            nc.vector.tensor_copy(sd[:], srow[:])
                            nc.sync.dma_start(dbg_d["s_0"][:], sd[:])
                            nc.sync.dma_start(dbg_d["num_1"][:], numn[:])
                            nc.sync.dma_start(dbg_d["yn2_1"][:], yn2n[:])

                # ---------------- final (per-class pipeline) ----------------
                dspF = sb2.tile([128, C, QL], DT, tag="dsp")
                nc.vector.scalar_tensor_tensor(dspF[:].rearrange("p c q -> p (c q)"),
                                               r_t[:], -2.0, dd1[:], op0=MUL, op1=ADD)
                hvF = sb.tile([QL, CD], F32, tag="hvF")
                n2q = sb2.tile([QL, C], F32, tag="n2q")
                fpsA = ps2.tile([QL, D + 1], F32, tag="nh")
                fpsB = ps2.tile([QL, D + 1], F32, tag="crhv")
                scr2 = sb2.tile([QL, D], DT, tag="scr2")
                scr3 = sb2.tile([QL, D], DT, tag="scr3")
                for c in range(C):
                    fps = fpsA if c % 2 == 0 else fpsB
                    nc.tensor.matmul(fps[:], dspF[:, c, :], hat_mB[:, D * c:D * c + 154],
                                     start=True, stop=True)
                    sl = slice(D * c, D * (c + 1))
                    if c % 2 == 0:
                        nc.scalar.copy(hvF[:, sl], fps[:, 0:153])
                        nc.vector.tensor_tensor(scr2[:], hvF[:, sl], hvF[:, sl], op=MUL)
                        nc.vector.tensor_reduce(
                            n2q[:, c:c + 1],
                            scr2[:].rearrange("p (a d) -> p a d", a=1), axis=AX, op=ADD)
                    else:
                        nc.vector.tensor_copy(hvF[:, sl], fps[:, 0:153])
                        nc.scalar.activation(scr3[:], hvF[:, sl], SQF,
                                             accum_out=n2q[:, c:c + 1])
                fp1 = sb2.tile([QL, C], F32, tag="fp1")
                nc.vector.tensor_scalar(fp1[:], n2q[:], 1.0, None, op0=ADD)
                fr1 = sb2.tile([QL, C], F32, tag="fr1")
                nc.vector.reciprocal(fr1[:], fp1[:])
                fln = sb2.tile([QL, C], F32, tag="fln")
                nc.scalar.activation(fln[:], n2q[:], LN, bias=epsb[0:QL, :])
                fr2 = sb2.tile([QL, C], F32, tag="fr2")
                nc.scalar.activation(fr2[:], fln[:], EXP, scale=-0.5)
                fs1 = sb2.tile([QL, C], F32, tag="fs1")
                nc.vector.tensor_scalar(fs1[:], fr1[:], -1.0, 1.0, op0=MUL, op1=ADD)
                fs = sb2.tile([QL, C], F32, tag="fs")
                nc.vector.tensor_tensor(fs[:], fs1[:], fr2[:], op=MUL)
                outT = sb.tile([QL, CD], F32, tag="outT")
                for c in range(C):
                    nc.vector.tensor_scalar(outT[:, D * c:D * (c + 1)],
                                            hvF[:, D * c:D * (c + 1)],
                                            fs[:, c:c + 1], None, op0=MUL)
                nc.sync.dma_start(out_d[:], outT[:])
                if dbg:
                    nc.sync.dma_start(dbg_d["dspF"][:], dspF[:].rearrange("p c q -> p (c q)"))
                    nc.sync.dma_start(dbg_d["hvF"][:], hvF[:])

    # All activations use only {Ln, Exp, Square, Copy} (act func set 6); one
    # table load suffices (the default solver would insert ~15 reloads).
    def _single_act_table_load():
        inst = mybir.InstLoadActFuncSet(
            name=nc.get_next_instruction_name(), ins=[], outs=[],
            act_func_set_id=6,
        )
        inst.engine = mybir.EngineType.Activation
        nc.register_instruction(inst)
        for blk in nc.main_func.blocks:
            for idx, bi in enumerate(blk.instructions):
                if isinstance(bi, mybir.InstActivation):
                    blk.instructions.insert(idx, inst)
                    return
        raise AssertionError("no activation found")

    nc.insert_act_table_loads = _single_act_table_load
    nc.compile()
    return nc


_CACHE = {}
LAST_EXEC_NS = None
LAST_RESULTS = None


def _cast_io(x):
    if USE_BF16:
        import ml_dtypes
        return np.asarray(x, dtype=ml_dtypes.bfloat16)
    return np.asarray(x, dtype=np.float32)


def kernel(m, q, W, b):
    m = np.asarray(m, dtype=np.float32)
    q = np.asarray(q, dtype=np.float32)
    W = np.asarray(W, dtype=np.float32)
    b = np.asarray(b, dtype=np.float32)
    assert m.shape == (I, K) and q.shape == (NCORES * QL, K) and W.shape == (K, CD)

    with_bias = bool(np.any(b))
    dbg = bool(int(os.environ.get("KERNEL_DBG", "0")))
    key = ("v4", with_bias, USE_BF16, dbg)
    if key not in _CACHE:
        _CACHE[key] = build(with_bias, dbg)
    nc = _CACHE[key]

    Wp = np.zeros((K, NPAD), dtype=np.float32)
    Wp[:, :CD] = W
    Wsum = np.zeros((K, 8), dtype=np.float32)
    Wsum[:, :C] = W.reshape(K, C, D).sum(axis=2)
    mT = np.ascontiguousarray(m.T)
    eye = np.eye(128, dtype=np.float32)

    common = {"mT": _cast_io(mT), "Wp": _cast_io(Wp), "Wsum": _cast_io(Wsum),
              "eye": _cast_io(eye)}
    if with_bias:
        common["b"] = _cast_io(b.reshape(1, CD))
        common["bsum"] = _cast_io(
            np.pad(b.reshape(C, D).sum(axis=1), (0, 3)).reshape(1, 8))
    in_maps = []
    for i in range(NCORES):
        qT = np.ascontiguousarray(q[QL * i:QL * (i + 1)].T)
        in_maps.append({**common, "qT": _cast_io(qT)})

    res = run_bass_kernel_spmd(nc, in_maps, list(range(NCORES)))
    global LAST_EXEC_NS, LAST_RESULTS
    LAST_EXEC_NS = res.exec_time_ns
    LAST_RESULTS = res.results
    out = np.concatenate([np.asarray(res.results[i]["out"], dtype=np.float32)
                          for i in range(NCORES)], axis=0)
    return out


if __name__ == "__main__":
    rng = np.random.default_rng(0)
    m = rng.standard_normal((I, K)).astype(np.float32)
    q = rng.standard_normal((NCORES * QL, K)).astype(np.float32)
    W = (rng.standard_normal((K, CD)) * 0.02).astype(np.float32)
    b = np.zeros((CD,), dtype=np.float32)
    out = kernel(m=m, q=q, W=W, b=b)
    print("out", out.shape, out.dtype, np.abs(out).mean())
